# revision 6
# baseline (speedup 1.0000x reference)
"""Multi-head causal attention with relative position bias on 8 Trainium2
NeuronCores (Bass/Tile, SPMD).

Problem: B=1, S=4096, D=768, H=12 heads (hd=64).
  qkv = x @ Wqkv + bqkv ; per head: softmax(q k^T / 8 + rel_bias + causal) @ v
  out = attn_out @ Wout + bout

Sharding: query rows are interleaved round-robin across the 8 cores
(core c owns global rows c::8).  With row-interleaving every core's
kblock j only needs local queries i >= 16*j, so each core reads exactly
the lower-triangular half of its rel_bias slice (the dominant HBM
traffic), and the device program is identical across cores — only the
packed input data differs.

Device dataflow (all-transposed orientation, fp16 compute, f32 PSUM):
  QT/KT projections produce [head_dim, seq] layouts directly;
  scoresT[k,q] -> PSUM ; DVE adds the (DMA-transposed) bias -> fp16
  strip ; one ACT exp per 8-kblock strip ; AV matmuls accumulate
  attn_outT[d,q] with a ones-column in V providing softmax denominators;
  per-head 1/Z scaling via a DRAM-bounce partition broadcast; final
  Wout matmul + bout.
"""

import math
import os

import numpy as np

H = 12
NEG_SENTINEL = -60000.0  # masked-score value; exp() underflows to 0


# ----------------------------------------------------------------------------
# Walrus in this toolchain accepts at most one attached sem-wait per
# instruction; hoist extras onto standalone NoOps.
# ----------------------------------------------------------------------------

def _split_waits(nc, max_waits=1):
    import concourse.mybir as mybir
    n_split = 0
    for f in nc.m.functions:
        for blk in f.blocks:
            insts = blk.instructions
            new_insts = []
            for inst in insts:
                si = inst.sync_info
                if si is not None and len(si.on_wait) > max_waits:
                    extra = list(si.on_wait[: len(si.on_wait) - max_waits])
                    keep = list(si.on_wait[len(si.on_wait) - max_waits:])
                    for w in extra:
                        nop = mybir.InstNoOp(
                            name=f"I-waitfix-{nc.next_id()}",
                            engine=inst.engine,
                            sync_info=mybir.SyncInfo(on_wait=[w], on_update=[]),
                            text_hint="waitfix",
                            bass_nofuse=True,
                        )
                        new_insts.append(nop)
                        n_split += 1
                    si.on_wait = keep
                new_insts.append(inst)
            if len(new_insts) != len(insts):
                try:
                    blk.instructions = new_insts
                except Exception:
                    insts.clear()
                    insts.extend(new_insts)
    return n_split


# ----------------------------------------------------------------------------
# Device program builder (SPMD-uniform; per-core data differences live in the
# packed inputs, not the program).
# ----------------------------------------------------------------------------

def _widths(SQ, NJ):
    return [SQ - 16 * j for j in range(NJ)]


def _bias_rowoffs(heads, widths):
    offs = {}
    r = 0
    for h in range(heads):
        for j, w in enumerate(widths):
            offs[(h, j)] = r
            r += w
    return offs, r


def build_attention_nc(S=4096, D=768, heads=H, n_cores=8):
    import concourse.bass as bass
    import concourse.mybir as mybir
    import concourse.tile as tile

    FP16 = mybir.dt.float16
    F32 = mybir.dt.float32
    AF = mybir.ActivationFunctionType

    hd = 64
    assert D == heads * hd
    PAIRS = heads // 2
    DIN = D // 128          # contraction chunks of 128 (== PAIRS)
    SQ = S // n_cores       # query rows per core
    NJ = S // 128           # key blocks of 128
    ST = S // 512           # 512-wide s-tiles for KT projection
    QC = max(1, SQ // 128)  # 128-row query chunks for the final matmul
    QCP = min(128, SQ)      # partitions per final query chunk
    widths = _widths(SQ, NJ)
    rowoffs, bias_rows = _bias_rowoffs(heads, widths)
    groups = [list(range(g, min(g + 8, NJ))) for g in range(0, NJ, 8)]
    max_gw = max(sum(widths[j] for j in g) for g in groups)
    VCOL = NJ * 130         # vaug cols per pair: per kblock [vA(64)|1|vB(64)|1]

    nc = bass.Bass()
    xT = nc.dram_tensor("xT", [D, S], FP16, kind="ExternalInput")
    xqT = nc.dram_tensor("xqT", [D, SQ], FP16, kind="ExternalInput")
    wqkv = nc.dram_tensor("wqkv", [D, 3 * D], FP16, kind="ExternalInput")
    bq = nc.dram_tensor("bq", [128, DIN], F32, kind="ExternalInput")
    wout = nc.dram_tensor("wout", [D, D], FP16, kind="ExternalInput")
    boutp = nc.dram_tensor("boutp", [1, D], F32, kind="ExternalInput")
    biastri = nc.dram_tensor("biastri", [bias_rows, 128], FP16, kind="ExternalInput")
    out_c = nc.dram_tensor("out_c", [SQ, D], F32, kind="ExternalOutput")
    zbounce = nc.dram_tensor("zbounce", [heads, SQ], F32)

    with tile.TileContext(nc) as tc:
        with tc.tile_pool(name="resident", bufs=1) as res, \
             tc.tile_pool(name="ps_big", bufs=2, space="PSUM") as ps_big:

            # resident tiles --------------------------------------------------
            xt_sb = []
            for i in range(DIN):
                t = res.tile([128, S], FP16, name=f"xt{i}")
                nc.sync.dma_start(t[:], xT[128 * i:128 * (i + 1), :])
                xt_sb.append(t)
            wk_sb = []
            for i in range(DIN):
                t = res.tile([128, D], FP16, name=f"wk{i}")
                nc.sync.dma_start(t[:], wqkv[128 * i:128 * (i + 1), D:2 * D])
                wk_sb.append(t)
            wo_sb = []
            for i in range(DIN):
                t = res.tile([128, D], FP16, name=f"wo{i}")
                nc.sync.dma_start(t[:], wout[128 * i:128 * (i + 1), :])
                wo_sb.append(t)
            bq_sb = res.tile([128, DIN], F32, name="bq_sb")
            nc.sync.dma_start(bq_sb[:], bq[:, :])
            qt_sb = []
            for p in range(PAIRS):
                qt_sb.append(res.tile([128, SQ], FP16, name=f"qt{p}"))
            aot_sb = []
            for p in range(PAIRS):
                aot_sb.append(res.tile([128, SQ], FP16, name=f"aot{p}"))
            vaug = res.tile([128, PAIRS * VCOL], FP16, name="vaug")
            # ones columns: within each 65-wide half-block, col 64
            vaug_ones = vaug[:, :].rearrange("p (a r) -> p a r", r=65)[:, :, 64:65]
            nc.vector.memset(vaug_ones, 1.0)

            # early phase: QT projection + V projection ----------------------
            with tc.tile_pool(name="early", bufs=1) as early:
                xqt_sb = []
                for i in range(DIN):
                    t = early.tile([128, SQ], FP16, name=f"xqt{i}")
                    nc.sync.dma_start(t[:], xqT[128 * i:128 * (i + 1), :])
                    xqt_sb.append(t)
                wq_sb = []
                wv_sb = []
                for i in range(DIN):
                    t = early.tile([128, D], FP16, name=f"wq{i}")
                    nc.sync.dma_start(t[:], wqkv[128 * i:128 * (i + 1), 0:D])
                    wq_sb.append(t)
                    t2 = early.tile([128, D], FP16, name=f"wv{i}")
                    nc.sync.dma_start(t2[:], wqkv[128 * i:128 * (i + 1), 2 * D:3 * D])
                    wv_sb.append(t2)

                # QT: [D, SQ] in pair-stacked tiles
                for p in range(PAIRS):
                    psq = ps_big.tile([128, 768], F32, tag="big", name="psq")
                    for i in range(DIN):
                        nc.tensor.matmul(
                            psq[:, 0:SQ], wq_sb[i][:, 128 * p:128 * (p + 1)],
                            xqt_sb[i][:], start=(i == 0), stop=(i == DIN - 1))
                    nc.scalar.activation(qt_sb[p][:], psq[:, 0:SQ], AF.Identity,
                                         bias=bq_sb[:, p:p + 1])

                # V: natural [s, dout], scattered into vaug around the ones
                nd2 = min(512, D)
                for s in range(NJ):
                    psv = ps_big.tile([128, 768], F32, tag="big", name="psv")
                    for i in range(DIN):
                        nc.tensor.matmul(
                            psv[:, 0:nd2], xt_sb[i][:, 128 * s:128 * (s + 1)],
                            wv_sb[i][:, 0:nd2], start=(i == 0), stop=(i == DIN - 1))
                        if D > 512:
                            nc.tensor.matmul(
                                psv[:, 512:D], xt_sb[i][:, 128 * s:128 * (s + 1)],
                                wv_sb[i][:, 512:D], start=(i == 0),
                                stop=(i == DIN - 1))
                    # psum cols: head m at [64m, 64m+64); dest:
                    # vaug[:, VCOL*p + 130*s + 65*half : +64]
                    src3 = psv[:, 0:D].rearrange("p (A B c) -> p A B c",
                                                 A=PAIRS, B=2)
                    dst3 = vaug[:, :].rearrange("p (A B r) -> p A B r",
                                                A=PAIRS, r=65)
                    nc.vector.tensor_copy(dst3[:, :, 2 * s:2 * s + 2, 0:64], src3)

            # main phase ------------------------------------------------------
            with tc.tile_pool(name="mainp", bufs=1) as mainp, \
                 tc.tile_pool(name="ktp_pool", bufs=2) as ktp_pool, \
                 tc.tile_pool(name="strip_pool", bufs=3) as strip_pool, \
                 tc.tile_pool(name="bias_pool", bufs=6) as bias_pool, \
                 tc.tile_pool(name="avf_pool", bufs=2) as avf_pool, \
                 tc.tile_pool(name="rz_pool", bufs=2) as rz_pool, \
                 tc.tile_pool(name="rzb_pool", bufs=2) as rzb_pool, \
                 tc.tile_pool(name="outp_pool", bufs=2) as outp_pool, \
                 tc.tile_pool(name="ps_sc", bufs=3, space="PSUM") as ps_sc, \
                 tc.tile_pool(name="ps_av", bufs=1, space="PSUM") as ps_av:

                boutpb = mainp.tile([QCP, D], F32, name="boutpb")
                nc.sync.dma_start(boutpb[:], boutp[0:1, :].broadcast_to([QCP, D]))

                for p in range(PAIRS):
                    ktp = ktp_pool.tile([128, S], FP16, tag="kt", name="ktp")
                    for st in range(ST):
                        psk = ps_big.tile([128, 768], F32, tag="big", name="psk")
                        for i in range(DIN):
                            nc.tensor.matmul(
                                psk[:, 0:512],
                                wk_sb[i][:, 128 * p:128 * (p + 1)],
                                xt_sb[i][:, 512 * st:512 * (st + 1)],
                                start=(i == 0), stop=(i == DIN - 1))
                        nc.scalar.activation(ktp[:, 512 * st:512 * (st + 1)],
                                             psk[:, 0:512], AF.Copy)

                    for hh in (0, 1):
                        h = 2 * p + hh
                        av = ps_av.tile([65, SQ], F32, tag="av", name="av")
                        for gi, js in enumerate(groups):
                            gw = sum(widths[j] for j in js)
                            strip = strip_pool.tile([128, max_gw], FP16,
                                                    tag="strip", name="strip")
                            off = 0
                            for j in js:
                                W = widths[j]
                                pss = ps_sc.tile([128, 512], F32, tag="sc",
                                                 name="pss")
                                nc.tensor.matmul(
                                    pss[:, 0:W],
                                    ktp[64 * hh:64 * hh + 64,
                                        128 * j:128 * (j + 1)],
                                    qt_sb[p][64 * hh:64 * hh + 64, 16 * j:SQ],
                                    start=True, stop=True)
                                bt = bias_pool.tile([128, 512], FP16,
                                                    tag="biasb", name="bt")
                                r0 = rowoffs[(h, j)]
                                nc.sync.dma_start_transpose(
                                    bt[:, 0:W], biastri[r0:r0 + W, :])
                                nc.vector.tensor_tensor(
                                    strip[:, off:off + W], pss[:, 0:W],
                                    bt[:, 0:W], op=mybir.AluOpType.add)
                                off += W
                            nc.scalar.activation(strip[:, 0:gw], strip[:, 0:gw],
                                                 AF.Exp)
                            off = 0
                            for j in js:
                                W = widths[j]
                                nc.tensor.matmul(
                                    av[:, 16 * j:SQ],
                                    vaug[:, VCOL * p + 130 * j + 65 * hh:
                                         VCOL * p + 130 * j + 65 * hh + 65],
                                    strip[:, off:off + W],
                                    start=(j == 0), stop=(j == NJ - 1))
                                off += W
                        # epilogue: 1/Z scaling via DRAM-bounce broadcast
                        avf = avf_pool.tile([65, SQ], F32, tag="avf", name="avf")
                        nc.scalar.activation(avf[:], av[:], AF.Copy)
                        rz = rz_pool.tile([1, SQ], F32, tag="rz", name="rz")
                        nc.vector.reciprocal(rz[:], avf[64:65, :])
                        nc.sync.dma_start(zbounce[h:h + 1, :], rz[:])
                        rzb = rzb_pool.tile([64, SQ], F32, tag="rzb", name="rzb")
                        nc.sync.dma_start(
                            rzb[:], zbounce[h:h + 1, :].broadcast_to([64, SQ]))
                        nc.vector.tensor_tensor(
                            aot_sb[p][64 * hh:64 * hh + 64, :], avf[0:64, :],
                            rzb[:], op=mybir.AluOpType.mult)

                # final Wout + bout
                nd2 = min(512, D)
                for qc in range(QC):
                    pso = ps_big.tile([128, 768], F32, tag="big", name="pso")
                    for i in range(DIN):
                        nc.tensor.matmul(
                            pso[0:QCP, 0:nd2],
                            aot_sb[i][:, QCP * qc:QCP * (qc + 1)],
                            wo_sb[i][:, 0:nd2], start=(i == 0),
                            stop=(i == DIN - 1))
                        if D > 512:
                            nc.tensor.matmul(
                                pso[0:QCP, 512:D],
                                aot_sb[i][:, QCP * qc:QCP * (qc + 1)],
                                wo_sb[i][:, 512:D],
                                start=(i == 0), stop=(i == DIN - 1))
                    out_t = outp_pool.tile([QCP, D], F32, tag="outp",
                                           name="out_t")
                    nc.vector.tensor_tensor(out_t[:], pso[0:QCP, 0:D],
                                            boutpb[:], op=mybir.AluOpType.add)
                    nc.sync.dma_start(out_c[QCP * qc:QCP * (qc + 1), :],
                                      out_t[:])

    _split_waits(nc)
    return nc


# ----------------------------------------------------------------------------
# Host-side packing
# ----------------------------------------------------------------------------

def _pack_core_bias(rel_bias, causal_mask, c, S, heads, n_cores, widths,
                    bias_rows):
    """Pack core c's lower-triangular bias blocks (natural [q, k] layout,
    causal corners folded in) into a [bias_rows, 128] fp16 array."""
    SQ = S // n_cores
    NJ = S // 128
    out = np.empty((bias_rows, 128), dtype=np.float16)
    r = 0
    A = rel_bias[:, c::n_cores, :]  # this core's query rows (view)
    for h in range(heads):
        Ah = A[h]
        for j in range(NJ):
            W = widths[j]
            blk = np.array(Ah[16 * j:SQ, 128 * j:128 * (j + 1)],
                           dtype=np.float32)
            # rows i in [16j, 16j+16) are partially masked by causality
            gsl = slice(n_cores * 16 * j + c, n_cores * (16 * j + 16) + c,
                        n_cores)
            corner = np.asarray(causal_mask[gsl, 128 * j:128 * (j + 1)],
                                np.float32)
            blk[0:16, :] = blk[0:16, :] + np.where(corner < -1e8, NEG_SENTINEL,
                                                   corner)
            out[r:r + W, :] = blk.astype(np.float16)
            r += W
    assert r == bias_rows
    return out


def _prep_shared(x, Wqkv, bqkv, Wout, bout):
    D = x.shape[-1]
    Wq = np.asarray(Wqkv, np.float32).copy()
    Wq[:, 0:D] *= 0.125                     # fold 1/sqrt(hd) into the Q path
    wqkv16 = Wq.astype(np.float16)
    bqs = np.asarray(bqkv[0:D], np.float32) * 0.125
    DIN = D // 128
    bq_t = np.ascontiguousarray(bqs.reshape(DIN, 128).T)   # [128, DIN]
    bv = np.asarray(bqkv[2 * D:3 * D], np.float32)
    boutp = (bv @ np.asarray(Wout, np.float32)
             + np.asarray(bout, np.float32)).reshape(1, D).astype(np.float32)
    xT16 = np.ascontiguousarray(
        np.asarray(x[0], np.float32).T).astype(np.float16)
    wout16 = np.asarray(Wout, np.float32).astype(np.float16)
    return xT16, wqkv16, bq_t, wout16, boutp


def _is_causal(causal_mask):
    m = np.asarray(causal_mask)
    S = m.shape[0]
    unmasked = m > -1e8
    if not np.array_equal(unmasked, np.tril(np.ones((S, S), dtype=bool))):
        return False
    return bool(np.all(np.where(unmasked, m, 0.0) == 0.0))


def _reference_numpy(x, Wqkv, bqkv, Wout, bout, rel_bias, causal_mask):
    B, S, D = x.shape
    heads = rel_bias.shape[0]
    hd = D // heads
    x2 = np.asarray(x[0], np.float64)
    qkv = x2 @ np.asarray(Wqkv, np.float64) + np.asarray(bqkv, np.float64)
    q, k, v = np.split(qkv, 3, axis=-1)
    out = np.empty((S, D), np.float64)
    for h in range(heads):
        qh = q[:, h * hd:(h + 1) * hd]
        kh = k[:, h * hd:(h + 1) * hd]
        vh = v[:, h * hd:(h + 1) * hd]
        s = qh @ kh.T / math.sqrt(hd)
        s += np.asarray(rel_bias[h], np.float64) + np.asarray(causal_mask,
                                                              np.float64)
        s -= s.max(axis=-1, keepdims=True)
        e = np.exp(s)
        a = e / e.sum(axis=-1, keepdims=True)
        out[:, h * hd:(h + 1) * hd] = a @ vh
    res = out @ np.asarray(Wout, np.float64) + np.asarray(bout, np.float64)
    return res[None].astype(np.float32)


_NC_CACHE = {}


def kernel(x, Wqkv, bqkv, Wout, bout, rel_bias, causal_mask):
    x = np.asarray(x)
    B, S, D = x.shape
    heads = rel_bias.shape[0]
    n_cores = 8

    if not _is_causal(causal_mask):
        return _reference_numpy(x, Wqkv, bqkv, Wout, bout, rel_bias,
                                causal_mask)

    from concourse.bass_utils import run_bass_kernel_spmd

    key = (S, D, heads, n_cores)
    if key not in _NC_CACHE:
        _NC_CACHE[key] = build_attention_nc(S=S, D=D, heads=heads,
                                            n_cores=n_cores)
    nc = _NC_CACHE[key]

    SQ = S // n_cores
    NJ = S // 128
    widths = _widths(SQ, NJ)
    _, bias_rows = _bias_rowoffs(heads, widths)

    xT16, wqkv16, bq_t, wout16, boutp = _prep_shared(x, Wqkv, bqkv, Wout, bout)

    in_maps = []
    for c in range(n_cores):
        xq = np.ascontiguousarray(np.asarray(x[0, c::n_cores, :],
                                             np.float32).T)
        in_maps.append({
            "xT": xT16,
            "xqT": xq.astype(np.float16),
            "wqkv": wqkv16,
            "bq": bq_t,
            "wout": wout16,
            "boutp": boutp,
            "biastri": _pack_core_bias(rel_bias, causal_mask, c, S, heads,
                                       n_cores, widths, bias_rows),
        })

    trace = os.environ.get("ATTN_KERNEL_TRACE", "0") == "1"
    res = run_bass_kernel_spmd(nc, in_maps, list(range(n_cores)), trace=trace)
    globals()["LAST_RESULTS"] = res

    out = np.empty((S, D), dtype=np.float32)
    for c in range(n_cores):
        out[c::n_cores, :] = res.results[c]["out_c"]
    return out[None]


# revision 10
# speedup vs baseline: 1.6041x; 1.6041x over previous
"""Multi-head causal attention with relative position bias on 8 Trainium2
NeuronCores (Bass/Tile, SPMD).

Problem: B=1, S=4096, D=768, H=12 heads (hd=64).
  qkv = x @ Wqkv + bqkv ; per head: softmax(q k^T / 8 + rel_bias + causal) @ v
  out = attn_out @ Wout + bout

Sharding: query rows are interleaved round-robin across the 8 cores
(core c owns global rows c::8).  With row-interleaving every core's
kblock j only needs local queries i >= 16*j, so each core reads exactly
the lower-triangular half of its rel_bias slice (the dominant HBM
traffic), and the device program is identical across cores — only the
packed input data differs.

Device dataflow (all-transposed orientation, fp16 compute, f32 PSUM):
  QT/KT projections produce [head_dim, seq] layouts directly;
  scoresT[k,q] -> PSUM ; DVE adds the (DMA-transposed) bias -> fp16
  strip ; one ACT exp per 8-kblock strip ; AV matmuls accumulate
  attn_outT[d,q] with a ones-column in V providing softmax denominators;
  per-head 1/Z scaling via a DRAM-bounce partition broadcast; final
  Wout matmul + bout.
"""

import math
import os

import numpy as np

H = 12
NEG_SENTINEL = -60000.0  # masked-score value; exp() underflows to 0


# ----------------------------------------------------------------------------
# Walrus in this toolchain accepts at most one attached sem-wait per
# instruction; hoist extras onto standalone NoOps.
# ----------------------------------------------------------------------------

def _split_waits(nc, max_waits=1):
    import concourse.mybir as mybir
    n_split = 0
    for f in nc.m.functions:
        for blk in f.blocks:
            insts = blk.instructions
            new_insts = []
            for inst in insts:
                si = inst.sync_info
                if si is not None and len(si.on_wait) > max_waits:
                    extra = list(si.on_wait[: len(si.on_wait) - max_waits])
                    keep = list(si.on_wait[len(si.on_wait) - max_waits:])
                    for w in extra:
                        nop = mybir.InstNoOp(
                            name=f"I-waitfix-{nc.next_id()}",
                            engine=inst.engine,
                            sync_info=mybir.SyncInfo(on_wait=[w], on_update=[]),
                            text_hint="waitfix",
                            bass_nofuse=True,
                        )
                        new_insts.append(nop)
                        n_split += 1
                    si.on_wait = keep
                new_insts.append(inst)
            if len(new_insts) != len(insts):
                try:
                    blk.instructions = new_insts
                except Exception:
                    insts.clear()
                    insts.extend(new_insts)
    return n_split


# ----------------------------------------------------------------------------
# Device program builder (SPMD-uniform; per-core data differences live in the
# packed inputs, not the program).
# ----------------------------------------------------------------------------

def _widths(SQ, NJ):
    return [SQ - 16 * j for j in range(NJ)]


def _bias_rowoffs(heads, widths):
    """Element offsets of each (h, j) bias block in the flat packed tensor.
    Block (h, j) is stored pre-transposed as [128 k, W q], row-major."""
    offs = {}
    r = 0
    for h in range(heads):
        for j, w in enumerate(widths):
            offs[(h, j)] = r
            r += 128 * w
    return offs, r


def build_attention_nc(S=4096, D=768, heads=H, n_cores=8):
    import concourse.bass as bass
    import concourse.mybir as mybir
    import concourse.tile as tile

    FP16 = mybir.dt.float16
    F32 = mybir.dt.float32
    AF = mybir.ActivationFunctionType

    hd = 64
    assert D == heads * hd
    PAIRS = heads // 2
    DIN = D // 128          # contraction chunks of 128 (== PAIRS)
    SQ = S // n_cores       # query rows per core
    NJ = S // 128           # key blocks of 128
    ST = S // 512           # 512-wide s-tiles for KT projection
    QC = max(1, SQ // 128)  # 128-row query chunks for the final matmul
    QCP = min(128, SQ)      # partitions per final query chunk
    widths = _widths(SQ, NJ)
    rowoffs, bias_rows = _bias_rowoffs(heads, widths)
    groups = [list(range(g, min(g + 8, NJ))) for g in range(0, NJ, 8)]
    max_gw = max(sum(widths[j] for j in g) for g in groups)
    VCOL = NJ * 130         # vaug cols per pair: per kblock [vA(64)|1|vB(64)|1]

    nc = bass.Bass()
    xT = nc.dram_tensor("xT", [D, S], FP16, kind="ExternalInput")
    xqT = nc.dram_tensor("xqT", [D, SQ], FP16, kind="ExternalInput")
    wqkv = nc.dram_tensor("wqkv", [D, 3 * D], FP16, kind="ExternalInput")
    bq = nc.dram_tensor("bq", [128, DIN], F32, kind="ExternalInput")
    wout = nc.dram_tensor("wout", [D, D], FP16, kind="ExternalInput")
    boutp = nc.dram_tensor("boutp", [1, D], F32, kind="ExternalInput")
    biastri = nc.dram_tensor("biastri", [bias_rows], FP16, kind="ExternalInput")
    out_c = nc.dram_tensor("out_c", [SQ, D], F32, kind="ExternalOutput")
    zbounce = nc.dram_tensor("zbounce", [heads, SQ], F32)

    with tile.TileContext(nc) as tc:
        with tc.tile_pool(name="resident", bufs=1) as res, \
             tc.tile_pool(name="ps_big", bufs=2, space="PSUM") as ps_big:

            # resident tiles --------------------------------------------------
            xt_sb = []
            for i in range(DIN):
                t = res.tile([128, S], FP16, name=f"xt{i}")
                nc.sync.dma_start(t[:], xT[128 * i:128 * (i + 1), :])
                xt_sb.append(t)
            wk_sb = []
            for i in range(DIN):
                t = res.tile([128, D], FP16, name=f"wk{i}")
                nc.sync.dma_start(t[:], wqkv[128 * i:128 * (i + 1), D:2 * D])
                wk_sb.append(t)
            wo_sb = []
            for i in range(DIN):
                t = res.tile([128, D], FP16, name=f"wo{i}")
                nc.sync.dma_start(t[:], wout[128 * i:128 * (i + 1), :])
                wo_sb.append(t)
            bq_sb = res.tile([128, DIN], F32, name="bq_sb")
            nc.sync.dma_start(bq_sb[:], bq[:, :])
            qt_sb = []
            for p in range(PAIRS):
                qt_sb.append(res.tile([128, SQ], FP16, name=f"qt{p}"))
            aot_sb = []
            for p in range(PAIRS):
                aot_sb.append(res.tile([128, SQ], FP16, name=f"aot{p}"))
            vaug = res.tile([128, PAIRS * VCOL], FP16, name="vaug")
            # ones columns: within each 65-wide half-block, col 64
            vaug_ones = vaug[:, :].rearrange("p (a r) -> p a r", r=65)[:, :, 64:65]
            nc.vector.memset(vaug_ones, 1.0)

            # early phase: QT projection + V projection ----------------------
            with tc.tile_pool(name="early", bufs=1) as early:
                xqt_sb = []
                for i in range(DIN):
                    t = early.tile([128, SQ], FP16, name=f"xqt{i}")
                    nc.sync.dma_start(t[:], xqT[128 * i:128 * (i + 1), :])
                    xqt_sb.append(t)
                wq_sb = []
                wv_sb = []
                for i in range(DIN):
                    t = early.tile([128, D], FP16, name=f"wq{i}")
                    nc.sync.dma_start(t[:], wqkv[128 * i:128 * (i + 1), 0:D])
                    wq_sb.append(t)
                    t2 = early.tile([128, D], FP16, name=f"wv{i}")
                    nc.sync.dma_start(t2[:], wqkv[128 * i:128 * (i + 1), 2 * D:3 * D])
                    wv_sb.append(t2)

                # QT: [D, SQ] in pair-stacked tiles
                for p in range(PAIRS):
                    psq = ps_big.tile([128, 768], F32, tag="big", name="psq")
                    for i in range(DIN):
                        nc.tensor.matmul(
                            psq[:, 0:SQ], wq_sb[i][:, 128 * p:128 * (p + 1)],
                            xqt_sb[i][:], start=(i == 0), stop=(i == DIN - 1))
                    nc.scalar.activation(qt_sb[p][:], psq[:, 0:SQ], AF.Identity,
                                         bias=bq_sb[:, p:p + 1])

                # V: natural [s, dout], scattered into vaug around the ones
                nd2 = min(512, D)
                for s in range(NJ):
                    psv = ps_big.tile([128, 768], F32, tag="big", name="psv")
                    for i in range(DIN):
                        nc.tensor.matmul(
                            psv[:, 0:nd2], xt_sb[i][:, 128 * s:128 * (s + 1)],
                            wv_sb[i][:, 0:nd2], start=(i == 0), stop=(i == DIN - 1))
                        if D > 512:
                            nc.tensor.matmul(
                                psv[:, 512:D], xt_sb[i][:, 128 * s:128 * (s + 1)],
                                wv_sb[i][:, 512:D], start=(i == 0),
                                stop=(i == DIN - 1))
                    # psum cols: head m at [64m, 64m+64); dest:
                    # vaug[:, VCOL*p + 130*s + 65*half : +64]
                    src3 = psv[:, 0:D].rearrange("p (A B c) -> p A B c",
                                                 A=PAIRS, B=2)
                    dst3 = vaug[:, :].rearrange("p (A B r) -> p A B r",
                                                A=PAIRS, r=65)
                    nc.vector.tensor_copy(dst3[:, :, 2 * s:2 * s + 2, 0:64], src3)

            # main phase ------------------------------------------------------
            with tc.tile_pool(name="mainp", bufs=1) as mainp, \
                 tc.tile_pool(name="ktp_pool", bufs=2) as ktp_pool, \
                 tc.tile_pool(name="strip_pool", bufs=3) as strip_pool, \
                 tc.tile_pool(name="bias_pool", bufs=6) as bias_pool, \
                 tc.tile_pool(name="avf_pool", bufs=2) as avf_pool, \
                 tc.tile_pool(name="rz_pool", bufs=2) as rz_pool, \
                 tc.tile_pool(name="rzb_pool", bufs=2) as rzb_pool, \
                 tc.tile_pool(name="outp_pool", bufs=2) as outp_pool, \
                 tc.tile_pool(name="ps_sc", bufs=3, space="PSUM") as ps_sc, \
                 tc.tile_pool(name="ps_av", bufs=1, space="PSUM") as ps_av:

                boutpb = mainp.tile([QCP, D], F32, name="boutpb")
                nc.sync.dma_start(boutpb[:], boutp[0:1, :].broadcast_to([QCP, D]))

                for p in range(PAIRS):
                    ktp = ktp_pool.tile([128, S], FP16, tag="kt", name="ktp")
                    for st in range(ST):
                        psk = ps_big.tile([128, 768], F32, tag="big", name="psk")
                        for i in range(DIN):
                            nc.tensor.matmul(
                                psk[:, 0:512],
                                wk_sb[i][:, 128 * p:128 * (p + 1)],
                                xt_sb[i][:, 512 * st:512 * (st + 1)],
                                start=(i == 0), stop=(i == DIN - 1))
                        nc.scalar.activation(ktp[:, 512 * st:512 * (st + 1)],
                                             psk[:, 0:512], AF.Copy)

                    for hh in (0, 1):
                        h = 2 * p + hh
                        av = ps_av.tile([65, SQ], F32, tag="av", name="av")
                        for gi, js in enumerate(groups):
                            gw = sum(widths[j] for j in js)
                            strip = strip_pool.tile([128, max_gw], FP16,
                                                    tag="strip", name="strip")
                            off = 0
                            for j in js:
                                W = widths[j]
                                pss = ps_sc.tile([128, 512], F32, tag="sc",
                                                 name="pss")
                                nc.tensor.matmul(
                                    pss[:, 0:W],
                                    ktp[64 * hh:64 * hh + 64,
                                        128 * j:128 * (j + 1)],
                                    qt_sb[p][64 * hh:64 * hh + 64, 16 * j:SQ],
                                    start=True, stop=True)
                                bt = bias_pool.tile([128, 512], FP16,
                                                    tag="biasb", name="bt")
                                r0 = rowoffs[(h, j)]
                                src = biastri[r0:r0 + 128 * W].rearrange(
                                    "(p w) -> p w", w=W)
                                nc.sync.dma_start(bt[:, 0:W], src)
                                nc.vector.tensor_tensor(
                                    strip[:, off:off + W], pss[:, 0:W],
                                    bt[:, 0:W], op=mybir.AluOpType.add)
                                off += W
                            nc.scalar.activation(strip[:, 0:gw], strip[:, 0:gw],
                                                 AF.Exp)
                            off = 0
                            for j in js:
                                W = widths[j]
                                nc.tensor.matmul(
                                    av[:, 16 * j:SQ],
                                    vaug[:, VCOL * p + 130 * j + 65 * hh:
                                         VCOL * p + 130 * j + 65 * hh + 65],
                                    strip[:, off:off + W],
                                    start=(j == 0), stop=(j == NJ - 1))
                                off += W
                        # epilogue: 1/Z scaling via DRAM-bounce broadcast
                        avf = avf_pool.tile([65, SQ], F32, tag="avf", name="avf")
                        nc.scalar.activation(avf[:], av[:], AF.Copy)
                        rz = rz_pool.tile([1, SQ], F32, tag="rz", name="rz")
                        nc.vector.reciprocal(rz[:], avf[64:65, :])
                        nc.sync.dma_start(zbounce[h:h + 1, :], rz[:])
                        rzb = rzb_pool.tile([64, SQ], F32, tag="rzb", name="rzb")
                        nc.sync.dma_start(
                            rzb[:], zbounce[h:h + 1, :].broadcast_to([64, SQ]))
                        nc.vector.tensor_tensor(
                            aot_sb[p][64 * hh:64 * hh + 64, :], avf[0:64, :],
                            rzb[:], op=mybir.AluOpType.mult)

                # final Wout + bout
                nd2 = min(512, D)
                for qc in range(QC):
                    pso = ps_big.tile([128, 768], F32, tag="big", name="pso")
                    for i in range(DIN):
                        nc.tensor.matmul(
                            pso[0:QCP, 0:nd2],
                            aot_sb[i][:, QCP * qc:QCP * (qc + 1)],
                            wo_sb[i][:, 0:nd2], start=(i == 0),
                            stop=(i == DIN - 1))
                        if D > 512:
                            nc.tensor.matmul(
                                pso[0:QCP, 512:D],
                                aot_sb[i][:, QCP * qc:QCP * (qc + 1)],
                                wo_sb[i][:, 512:D],
                                start=(i == 0), stop=(i == DIN - 1))
                    out_t = outp_pool.tile([QCP, D], F32, tag="outp",
                                           name="out_t")
                    nc.vector.tensor_tensor(out_t[:], pso[0:QCP, 0:D],
                                            boutpb[:], op=mybir.AluOpType.add)
                    nc.sync.dma_start(out_c[QCP * qc:QCP * (qc + 1), :],
                                      out_t[:])

    _split_waits(nc)
    return nc


# ----------------------------------------------------------------------------
# Host-side packing
# ----------------------------------------------------------------------------

def _pack_core_bias(rel_bias, causal_mask, c, S, heads, n_cores, widths,
                    bias_rows):
    """Pack core c's lower-triangular bias blocks into a flat fp16 array.
    Block (h, j) is stored TRANSPOSED as [128 k, W q] row-major so the
    device can load it with a plain high-bandwidth DMA."""
    SQ = S // n_cores
    NJ = S // 128
    out = np.empty(bias_rows, dtype=np.float16)
    r = 0
    A = rel_bias[:, c::n_cores, :]  # this core's query rows (view)
    for h in range(heads):
        Ah = np.ascontiguousarray(A[h], dtype=np.float32)  # [SQ, S] row gather
        # fold causal corners in place: rows i in [16j, 16j+16) of block j
        for j in range(NJ):
            gsl = slice(n_cores * 16 * j + c, n_cores * (16 * j + 16) + c,
                        n_cores)
            corner = np.asarray(causal_mask[gsl, 128 * j:128 * (j + 1)],
                                np.float32)
            Ah[16 * j:16 * j + 16, 128 * j:128 * (j + 1)] += np.where(
                corner < -1e8, NEG_SENTINEL, corner)
        for j in range(NJ):
            W = widths[j]
            blk = Ah[16 * j:SQ, 128 * j:128 * (j + 1)]       # [W, 128]
            out[r:r + 128 * W] = blk.T.astype(np.float16).reshape(-1)
            r += 128 * W
    assert r == bias_rows
    return out


def _prep_shared(x, Wqkv, bqkv, Wout, bout):
    D = x.shape[-1]
    Wq = np.asarray(Wqkv, np.float32).copy()
    Wq[:, 0:D] *= 0.125                     # fold 1/sqrt(hd) into the Q path
    wqkv16 = Wq.astype(np.float16)
    bqs = np.asarray(bqkv[0:D], np.float32) * 0.125
    DIN = D // 128
    bq_t = np.ascontiguousarray(bqs.reshape(DIN, 128).T)   # [128, DIN]
    bv = np.asarray(bqkv[2 * D:3 * D], np.float32)
    boutp = (bv @ np.asarray(Wout, np.float32)
             + np.asarray(bout, np.float32)).reshape(1, D).astype(np.float32)
    xT16 = np.ascontiguousarray(
        np.asarray(x[0], np.float32).T).astype(np.float16)
    wout16 = np.asarray(Wout, np.float32).astype(np.float16)
    return xT16, wqkv16, bq_t, wout16, boutp


def _is_causal(causal_mask):
    m = np.asarray(causal_mask)
    S = m.shape[0]
    unmasked = m > -1e8
    if not np.array_equal(unmasked, np.tril(np.ones((S, S), dtype=bool))):
        return False
    return bool(np.all(np.where(unmasked, m, 0.0) == 0.0))


def _reference_numpy(x, Wqkv, bqkv, Wout, bout, rel_bias, causal_mask):
    B, S, D = x.shape
    heads = rel_bias.shape[0]
    hd = D // heads
    x2 = np.asarray(x[0], np.float64)
    qkv = x2 @ np.asarray(Wqkv, np.float64) + np.asarray(bqkv, np.float64)
    q, k, v = np.split(qkv, 3, axis=-1)
    out = np.empty((S, D), np.float64)
    for h in range(heads):
        qh = q[:, h * hd:(h + 1) * hd]
        kh = k[:, h * hd:(h + 1) * hd]
        vh = v[:, h * hd:(h + 1) * hd]
        s = qh @ kh.T / math.sqrt(hd)
        s += np.asarray(rel_bias[h], np.float64) + np.asarray(causal_mask,
                                                              np.float64)
        s -= s.max(axis=-1, keepdims=True)
        e = np.exp(s)
        a = e / e.sum(axis=-1, keepdims=True)
        out[:, h * hd:(h + 1) * hd] = a @ vh
    res = out @ np.asarray(Wout, np.float64) + np.asarray(bout, np.float64)
    return res[None].astype(np.float32)


_NC_CACHE = {}


def kernel(x, Wqkv, bqkv, Wout, bout, rel_bias, causal_mask):
    x = np.asarray(x)
    B, S, D = x.shape
    heads = rel_bias.shape[0]
    n_cores = 8

    if not _is_causal(causal_mask):
        return _reference_numpy(x, Wqkv, bqkv, Wout, bout, rel_bias,
                                causal_mask)

    from concourse.bass_utils import run_bass_kernel_spmd

    key = (S, D, heads, n_cores)
    if key not in _NC_CACHE:
        _NC_CACHE[key] = build_attention_nc(S=S, D=D, heads=heads,
                                            n_cores=n_cores)
    nc = _NC_CACHE[key]

    SQ = S // n_cores
    NJ = S // 128
    widths = _widths(SQ, NJ)
    _, bias_rows = _bias_rowoffs(heads, widths)

    xT16, wqkv16, bq_t, wout16, boutp = _prep_shared(x, Wqkv, bqkv, Wout, bout)

    in_maps = []
    for c in range(n_cores):
        xq = np.ascontiguousarray(np.asarray(x[0, c::n_cores, :],
                                             np.float32).T)
        in_maps.append({
            "xT": xT16,
            "xqT": xq.astype(np.float16),
            "wqkv": wqkv16,
            "bq": bq_t,
            "wout": wout16,
            "boutp": boutp,
            "biastri": _pack_core_bias(rel_bias, causal_mask, c, S, heads,
                                       n_cores, widths, bias_rows),
        })

    trace = os.environ.get("ATTN_KERNEL_TRACE", "0") == "1"
    res = run_bass_kernel_spmd(nc, in_maps, list(range(n_cores)), trace=trace)
    globals()["LAST_RESULTS"] = res

    out = np.empty((S, D), dtype=np.float32)
    for c in range(n_cores):
        out[c::n_cores, :] = res.results[c]["out_c"]
    return out[None]
